# revision 27
# baseline (speedup 1.0000x reference)
"""Longformer layer v2 (B=2, S=4096, D=768, H=12, w=128, NG=32) on 8 TRN2 cores.

Sharding: sequence-parallel. Core c owns tokens [q0, q0+1024) of batch b=c//4.
Each core computes band+global-key attention and the dense pipeline for its
1024 tokens; global-query rows emit flash partials combined on host.

v2 changes vs baseline:
- x arrives pre-transposed (feature-major) and pre-bf16 from the host: the
  on-device PE transpose phase is gone.
- All weights (Wq..Wo2) are DMA'd into resident SBUF tiles at kernel start so
  loads overlap early compute; Wo2 is no longer streamed twice.
- Attention processes heads in PAIRS. Head h=2p lives on partitions 0:64 of
  feature tile p, h=2p+1 on 64:128, so paired score matmuls land on disjoint
  PE row groups and run concurrently. Scores for both heads of a pair go to
  one 2-bank psum tile -> ONE exp ACT per key-chunk covers both heads.
- Global-query (pg) scores are extra columns of the band score psum (no
  separate exp); pg AV matmuls reuse the band AV stationary.
- Band AV accumulates per 512-query group [65,512] psum: the global-key AV
  (full span, start=True) runs first, the 6 clipped band contributions then
  pure-accumulate. 14 matmuls/head instead of 32.
- Softmax normalization per (head, group): one reciprocal [1,512]->bf16, a
  DMA partition-broadcast (gpsimd) to SBUF, copy+mul on DVE.
- LayerNorm: sums via col-tiled paired matmuls (s1/s2 in one bank), rstd/mr
  broadcast via gpsimd DMA, gain/bias applied with a fused tensor_scalar.
  Residuals ride in bf16; the residual adds are folded into the Wo/Wo2
  accumulation as identity matmuls.
"""

import math
import numpy as np
import ml_dtypes

import concourse.bacc as bacc
import concourse.mybir as mybir
import concourse.tile as tile
from concourse.masks import make_identity

F32 = mybir.dt.float32
BF16 = mybir.dt.bfloat16
AF = mybir.ActivationFunctionType
ALU = mybir.AluOpType

B, S, D, FF = 2, 4096, 768, 3072
H, DH, W, NG = 12, 64, 128, 32
EPS = 1e-12
T = 1024
TH = T + 2 * W
KD = D // 128       # 6
KF = FF // 128      # 24
NCH = T // W        # 8 owned chunks
NJ = NCH + 2        # key chunks jdx = 0..9 (j = jdx-1)
HALF = 512
N_CORES = 8
ISCALE = 1.0 / math.sqrt(DH)

# band window per key chunk j: query chunks [j-1, j, j+1] clipped to [0, 8)
WIN = {}
for j in range(-1, NCH + 1):
    cs = [c for c in (j - 1, j, j + 1) if 0 <= c < NCH]
    WIN[j] = (cs[0] * 128, len(cs) * 128)   # (q_lo, wj)

_nc_cache = {}


def build_body(nc, tc, ap, ctx, sim_mode=False):
    import os
    stop_after = os.environ.get("K2_STOP", "")
    ablate = set(os.environ.get("K2_ABLATE", "").split(","))
    mul_mask = os.environ.get("K2_MULMASK", "1") == "1"
    gp_dma = nc.sync if os.environ.get("K2_SYNC_DMA") else nc.gpsimd
    gp_mul = nc.gpsimd if os.environ.get("K2_POOL_MASK") else nc.vector
    # reciprocal_approx_fast measured broken on HW (custom-DVE op returns
    # garbage under this toolchain) - use the standard reciprocal.
    slow_recip = True
    gelu_f = AF.Identity if sim_mode else AF.Gelu
    persist = ctx.enter_context(tc.tile_pool(name="persist", bufs=1))

    def dummy_out(pool, og_too=True):
        z = pool.tile([128, T], F32, tag="zdum", name="zdum")
        nc.vector.memset(z, 0.0)
        for k in range(KD):
            nc.sync.dma_start(out=ap["outT"][k * 128:(k + 1) * 128, :], in_=z)
        if og_too:
            zg = pool.tile([65, NG], F32, tag="zgdum", name="zgdum")
            nc.vector.memset(zg, 1.0)
            for h in range(H):
                nc.sync.dma_start(out=ap["og"][h], in_=zg)

    # Resident weights for the late phases (Wo/Wi/Wo2). Tiles allocated here;
    # their DMAs are issued after the first-needed loads (x, Wq/Wk/Wv) so the
    # DMA queues serve the projection phase first. Wo lives in its own stack
    # so its SBUF frees before the FFN phase.
    wo = []  # allocated in wo_pool below (freed before the FFN phase)
    wi = [persist.tile([128, FF], BF16, tag=f"wi{k}", name=f"wi{k}")
          for k in range(KD)]
    wo2 = [persist.tile([128, D], BF16, tag=f"wo2_{f}", name=f"wo2_{f}")
           for f in range(KF)]

    def load_late_weights():
        for k in range(KD):
            gp_dma.dma_start(out=wi[k],
                                in_=ap["Wi"][k * 128:(k + 1) * 128, :])
        for f in range(KF):
            gp_dma.dma_start(out=wo2[f],
                                in_=ap["Wo2"][f * 128:(f + 1) * 128, :])

    # ---------------- constants / biases ----------------
    identB = persist.tile([128, 128], BF16, tag="identB", name="identB")
    make_identity(nc, identB)
    ones_col = persist.tile([128, 1], BF16, tag="ones_col", name="ones_col")
    nc.vector.memset(ones_col, 1.0)
    ones_row = persist.tile([1, 128], BF16, tag="ones_row", name="ones_row")
    nc.vector.memset(ones_row, 1.0)
    eps_sc = persist.tile([1, 1], F32, tag="eps_sc", name="eps_sc")
    nc.vector.memset(eps_sc, EPS)

    def load_cols(name, n):
        t = persist.tile([128, n], F32, tag=name)
        nc.sync.dma_start(out=t, in_=ap[name].rearrange("(k p) -> p k", p=128))
        return t

    bq_sb = load_cols("bq", KD)
    bk_sb = load_cols("bk", KD)
    bo_sb = load_cols("bo", KD)
    bi_sb = load_cols("bi", KF)
    bo2_sb = load_cols("bo2", KD)
    g1_sb = load_cols("ln1_g", KD)
    b1_sb = load_cols("ln1_b", KD)
    g2_sb = load_cols("ln2_g", KD)
    b2_sb = load_cols("ln2_b", KD)

    bv_bc = persist.tile([128, D], BF16, tag="bv_bc", name="bv_bc")
    nc.gpsimd.dma_start(out=bv_bc, in_=ap["bv"].unsqueeze(0).partition_broadcast(128))

    am_sb = persist.tile([128, NJ], F32, tag="am_sb", name="am_sb")
    nc.sync.dma_start(out=am_sb, in_=ap["am_halo"].rearrange("(k p) -> p k", p=128))
    amg_sb = persist.tile([64, 1], F32, tag="amg_sb", name="amg_sb")
    nc.sync.dma_start(out=amg_sb, in_=ap["am_glob2"].unsqueeze(1))

    # ---------------- LayerNorm (feature-major, bf16 residual stream) ------
    def layernorm(u_tiles, cols, g_sb, b_sb, dest_aps, pools):
        """u_tiles: KD bf16 [128, cols] SBUF tiles. dest_aps[k]: [128, cols]."""
        usq_sb, row_sb, bcast_sb, s_ps, bc_ps = pools
        s = s_ps.tile([33, HALF], F32, tag="s", name="s")
        for k in range(KD):
            usq = usq_sb.tile([128, HALF], BF16, tag=f"usq{k % 2}",
                              name=f"usq{k % 2}", bufs=2)
            nc.vector.tensor_mul(out=usq[:, :cols], in0=u_tiles[k][:, :cols],
                                 in1=u_tiles[k][:, :cols])
            nc.tensor.matmul(s[0:1, :cols], ones_col, u_tiles[k][:, :cols],
                             start=(k == 0), stop=(k == KD - 1),
                             tile_position=(0, 0))
            nc.tensor.matmul(s[32:33, :cols], ones_col, usq[:, :cols],
                             start=(k == 0), stop=(k == KD - 1),
                             tile_position=(0, 32))
        mu = row_sb.tile([1, HALF], F32, tag="mu", name="mu")
        nc.vector.tensor_scalar_mul(out=mu[:, :cols], in0=s[0:1, :cols],
                                    scalar1=1.0 / D)
        q = row_sb.tile([1, HALF], F32, tag="q", name="q")
        nc.vector.tensor_mul(out=q[:, :cols], in0=s[0:1, :cols], in1=mu[:, :cols])
        vD = row_sb.tile([1, HALF], F32, tag="vD", name="vD")
        nc.vector.tensor_sub(out=vD[:, :cols], in0=s[32:33, :cols], in1=q[:, :cols])
        sd = row_sb.tile([1, HALF], F32, tag="sd", name="sd")
        nc.scalar.activation(out=sd[:, :cols], in_=vD[:, :cols], func=AF.Sqrt,
                             bias=eps_sc[:], scale=1.0 / D)
        rstd = row_sb.tile([1, HALF], BF16, tag="rstd", name="rstd")
        with nc.allow_low_precision(reason="bf16 norm scales"):
            nc.vector.reciprocal(out=rstd[:, :cols], in_=sd[:, :cols])
            mr = row_sb.tile([1, HALF], BF16, tag="mr", name="mr")
            nc.vector.tensor_mul(out=mr[:, :cols], in0=mu[:, :cols],
                                 in1=rstd[:, :cols])
        # broadcast via PE outer product, then park in SBUF bf16 so the
        # per-k muls run at DVE 2x rate (psum operands would force 1x)
        rb_ps = bc_ps.tile([128, HALF], F32, tag="rstd_bc", name="rstd_bc")
        nc.tensor.matmul(rb_ps[:, :cols], ones_row, rstd[:, :cols],
                         start=True, stop=True)
        mb_ps = bc_ps.tile([128, HALF], F32, tag="mr_bc", name="mr_bc")
        nc.tensor.matmul(mb_ps[:, :cols], ones_row, mr[:, :cols],
                         start=True, stop=True)
        rstd_bc = bcast_sb.tile([128, HALF], BF16, tag="rb", name="rb")
        nc.scalar.activation(out=rstd_bc[:, :cols], in_=rb_ps[:, :cols],
                             func=AF.Copy)
        mr_bc = bcast_sb.tile([128, HALF], BF16, tag="mb", name="mb")
        nc.scalar.activation(out=mr_bc[:, :cols], in_=mb_ps[:, :cols],
                             func=AF.Copy)
        for k in range(KD):
            w = usq_sb.tile([128, HALF], BF16, tag=f"w{k % 2}",
                            name=f"w{k % 2}", bufs=2)
            nc.vector.tensor_mul(out=w[:, :cols], in0=u_tiles[k][:, :cols],
                                 in1=rstd_bc[:, :cols])
            nc.vector.tensor_sub(out=w[:, :cols], in0=w[:, :cols],
                                 in1=mr_bc[:, :cols])
            with nc.allow_low_precision(reason="bf16 ln out"):
                nc.vector.tensor_scalar(
                    out=dest_aps[k], in0=w[:, :cols],
                    scalar1=g_sb[:, k:k + 1], scalar2=b_sb[:, k:k + 1],
                    op0=ALU.mult, op1=ALU.add)

    # attn_outB / u2 outlive inner scopes
    pool_ao = ctx.enter_context(tc.tile_pool(name="pool_ao", bufs=1))
    # split per column-half so FFN(half0) doesn't wait on LN1(half1)
    attn_outB = {(k, c0): pool_ao.tile([128, HALF], BF16, tag=f"aob{k}_{c0}",
                                       name=f"aob{k}_{c0}")
                 for k in range(KD) for c0 in (0, HALF)}

    with tc.tile_pool(name="pool_x", bufs=1) as pool_x, \
         tc.tile_pool(name="pool_ctx", bufs=1) as pool_ctx:
        xT = [pool_x.tile([128, TH], BF16, tag=f"xT{k}", name=f"xT{k}")
              for k in range(KD)]
        xgT = [pool_x.tile([128, NG], BF16, tag=f"xgT{k}", name=f"xgT{k}")
               for k in range(KD)]
        # first x column-chunk lands fast so projections start early
        XC = 640
        for k in range(KD):
            nc.sync.dma_start(out=xT[k][:, :XC],
                              in_=ap["xT"][k * 128:(k + 1) * 128, :XC])
            gp_dma.dma_start(out=xgT[k],
                                in_=ap["xgT"][k * 128:(k + 1) * 128, :])
        ctx_raw = [pool_ctx.tile([128, T], BF16, tag=f"ctx{k}", name=f"ctx{k}")
                   for k in range(KD)]

        with tc.tile_pool(name="pool_qkv", bufs=1) as pool_qkv:
            # v: token-major per halo chunk, heads interleaved with ones col
            v_sb = [pool_qkv.tile([128, H * 65], BF16, tag=f"v{j}", name=f"v{j}")
                    for j in range(NJ)]
            vg2 = pool_qkv.tile([64, H * 65], BF16, tag="vg2", name="vg2")
            # q/k tiles rotate per head-pair (projection is fused into the
            # attention pair loop so PE-bound proj hides ACT/DVE-bound attn)
            qTr = [pool_qkv.tile([128, T], BF16, tag=f"q{i}", name=f"q{i}")
                   for i in range(2)]
            kTr = [pool_qkv.tile([128, TH], BF16, tag=f"k{i}", name=f"k{i}")
                   for i in range(2)]
            qgr = [pool_qkv.tile([128, NG], BF16, tag=f"qg{i}", name=f"qg{i}")
                   for i in range(2)]
            kgr = [pool_qkv.tile([128, NG], BF16, tag=f"kg{i}", name=f"kg{i}")
                   for i in range(2)]

            with tc.tile_pool(name="wqkv", bufs=1) as wqkv_pool, \
                 tc.tile_pool(name="mask_sb", bufs=1) as mask_sb_pool, \
                 tc.tile_pool(name="esb", bufs=1) as esb, \
                 tc.tile_pool(name="egsb", bufs=1) as egsb, \
                 tc.tile_pool(name="rcsb", bufs=1) as rcsb, \
                 tc.tile_pool(name="bcsb", bufs=1) as bcsb, \
                 tc.tile_pool(name="ogsb", bufs=2) as ogsb:
                wq = [wqkv_pool.tile([128, D], BF16, tag=f"wq{k}",
                                     name=f"wq{k}") for k in range(KD)]
                wk = [wqkv_pool.tile([128, D], BF16, tag=f"wk{k}",
                                     name=f"wk{k}") for k in range(KD)]
                mask_sb = mask_sb_pool.tile([128, NJ, 3 * W], BF16, tag="mask",
                                            name="mask")
                gp_dma.dma_start(out=mask_sb, in_=ap["mask_all"])

                # ---- v projection (whole halo), before the pair loop ----
                with tc.tile_pool(name="wv_pool", bufs=1) as wv_pool, \
                     tc.tile_pool(name="vtmp_sb", bufs=2) as vtmp_sb, \
                     tc.tile_pool(name="vproj_ps", bufs=2,
                                  space="PSUM") as vproj_ps:
                    wv = [wv_pool.tile([128, D], BF16, tag=f"wv{k}",
                                       name=f"wv{k}") for k in range(KD)]
                    for k in range(KD):
                        nc.sync.dma_start(out=wv[k],
                                          in_=ap["Wv"][k * 128:(k + 1) * 128, :])
                    for k in range(KD):
                        nc.sync.dma_start(out=xT[k][:, XC:],
                                          in_=ap["xT"][k * 128:(k + 1) * 128,
                                                       XC:])
                        nc.sync.dma_start(out=wq[k],
                                          in_=ap["Wq"][k * 128:(k + 1) * 128, :])
                        nc.sync.dma_start(out=wk[k],
                                          in_=ap["Wk"][k * 128:(k + 1) * 128, :])
                    load_late_weights()
                    def v_project(src_tiles, n_tok, dest):
                        ps = vproj_ps.tile([128, D], F32, tag="vproj",
                                           name="vproj")
                        for c0 in range(0, D, HALF):
                            cw = min(HALF, D - c0)
                            for k in range(KD):
                                nc.tensor.matmul(ps[:n_tok, c0:c0 + cw],
                                                 src_tiles[k],
                                                 wv[k][:, c0:c0 + cw],
                                                 start=(k == 0),
                                                 stop=(k == KD - 1))
                        tmp = vtmp_sb.tile([128, D], F32, tag="vtmp",
                                           name="vtmp")
                        nc.vector.tensor_add(out=tmp[:n_tok], in0=ps[:n_tok],
                                             in1=bv_bc[:n_tok])
                        dv = dest.rearrange("p (h e) -> p h e", e=65)[:n_tok]
                        nc.vector.tensor_copy(
                            out=dv[:, :, 0:64],
                            in_=tmp[:n_tok].rearrange("p (h d) -> p h d", d=DH))
                        nc.vector.memset(dv[:, :, 64:65], 1.0)

                    for j in range(NJ):
                        v_project([xT[k][:, j * 128:(j + 1) * 128]
                                   for k in range(KD)], 128, v_sb[j])
                    v_project(xgT, NG, vg2)
                    nc.vector.tensor_copy(out=vg2[32:64], in_=vg2[0:32])

                if stop_after == "proj":
                    dummy_out(pool_qkv)
                    return

                def kslice(tiles, h, cols):
                    return tiles[(h % 2) * DH:(h % 2) * DH + DH, cols]

                import os as _os
                with tc.tile_pool(name="proj_ps",
                                  bufs=int(_os.environ.get("K2_PROJB", "2")),
                                  space="PSUM") as proj_ps, \
                     tc.tile_pool(name="sc_ps",
                                  bufs=int(_os.environ.get("K2_SCB", "1")),
                                  space="PSUM") as sc_ps, \
                     tc.tile_pool(name="av_ps",
                                  bufs=int(_os.environ.get("K2_AVB", "3")),
                                  space="PSUM") as av_ps, \
                     tc.tile_pool(name="pg_ps", bufs=1,
                                  space="PSUM") as pg_ps:
                    for p in range(H // 2):
                        h0, h1 = 2 * p, 2 * p + 1
                        pp = p % 2
                        qT, kT, qgT, kgT = qTr[pp], kTr[pp], qgr[pp], kgr[pp]
                        # ---- q/k projections for this pair (feature tile p);
                        # psum->sbuf copies ride on DVE with fused bias ----
                        for wt, bias_sb, dest, gdest, ncols, coff in (
                                (wq, bq_sb, qT, qgT, T, W),
                                (wk, bk_sb, kT, kgT, TH, 0)):
                            pcp = os.environ.get("K2_PROJCOPY", "dve")
                            def proj_copy(dst, src, bias):
                                if pcp == "act":
                                    nc.scalar.activation(out=dst, in_=src,
                                                         func=AF.Identity,
                                                         bias=bias)
                                elif pcp == "pool":
                                    nc.gpsimd.tensor_scalar_add(
                                        out=dst, in0=src, scalar1=bias)
                                else:
                                    nc.vector.tensor_scalar_add(
                                        out=dst, in0=src, scalar1=bias)
                            for c0 in range(0, ncols, HALF):
                                cw = min(HALF, ncols - c0)
                                ps = proj_ps.tile([128, HALF], F32, tag="proj",
                                                  name="proj")
                                for k in range(KD):
                                    nc.tensor.matmul(
                                        ps[:, :cw],
                                        wt[k][:, p * 128:(p + 1) * 128],
                                        xT[k][:, coff + c0:coff + c0 + cw],
                                        start=(k == 0), stop=(k == KD - 1))
                                proj_copy(dest[:, c0:c0 + cw], ps[:, :cw],
                                          bias_sb[:, p:p + 1])
                            psg = proj_ps.tile([128, NG], F32, tag="proj",
                                               name="psg")
                            for k in range(KD):
                                nc.tensor.matmul(psg[:],
                                                 wt[k][:, p * 128:(p + 1) * 128],
                                                 xgT[k], start=(k == 0),
                                                 stop=(k == KD - 1))
                            proj_copy(gdest[:], psg[:], bias_sb[:, p:p + 1])

                        # ---- global-key scores (psum borrowed from proj) ----
                        eg = egsb.tile([64, T], BF16, tag="eg", name="eg")
                        for c0 in range(0, T, HALF):
                            gps = proj_ps.tile([64, HALF], F32, tag="proj",
                                               name="gps")
                            # pending-zero tracking is per-partition: each
                            # head's MM owns its own partition range
                            nc.tensor.matmul(gps[0:32, :],
                                             kslice(kgT, h0, slice(0, NG)),
                                             kslice(qT, h0, slice(c0, c0 + HALF)),
                                             start=True, stop=True)
                            nc.tensor.matmul(gps[32:64, :],
                                             kslice(kgT, h1, slice(0, NG)),
                                             kslice(qT, h1, slice(c0, c0 + HALF)),
                                             start=True, stop=True)
                            nc.scalar.activation(out=eg[:, c0:c0 + HALF],
                                                 in_=gps, func=AF.Exp,
                                                 bias=amg_sb[:], scale=ISCALE)

                        e_tiles = {}

                        def emit_scores(jlist):
                            for j in jlist:
                                jdx = j + 1
                                q_lo, wj = WIN[j]
                                has_pg = 1 <= jdx <= NCH and "nopg" not in ablate
                                wtot = wj + (NG if has_pg else 0)
                                ps = sc_ps.tile([128, 2, HALF], F32, tag="sc",
                                                name="sc")
                                # band-validity mask folded in as an additive
                                # identity matmul (values 0 / -1000) so exp
                                # output feeds PE directly with no post-exp
                                # DVE/Pool multiply on the critical chain;
                                # back-to-back adds share the identB load
                                mms = []
                                for hp, h in ((0, h0), (1, h1)):
                                    kst = kslice(kT, h, slice(jdx * 128,
                                                              jdx * 128 + 128))
                                    mms.append((ps[:, hp, :wj], kst,
                                                kslice(qT, h,
                                                       slice(q_lo, q_lo + wj)),
                                                True))
                                    if has_pg:
                                        mms.append((ps[:, hp, wj:wj + NG], kst,
                                                    kslice(qgT, h,
                                                           slice(0, NG)),
                                                    False))
                                if "nomask" not in ablate and not mul_mask:
                                    mms += [(ps[:, hp, :wj], identB,
                                             mask_sb[:, jdx, :wj], False)
                                            for hp in (0, 1)]
                                for i, (o, st, mv, first) in enumerate(mms):
                                    nc.tensor.matmul(o, st, mv, start=first,
                                                     stop=(i == len(mms) - 1))
                                et = esb.tile([128, 2, 416], BF16,
                                              tag=f"e{jdx}", name=f"e{jdx}")
                                nc.scalar.activation(out=et[:, :, :wtot],
                                                     in_=ps[:, :, :wtot],
                                                     func=(AF.Copy if "noexp"
                                                           in ablate
                                                           else AF.Exp),
                                                     bias=am_sb[:, jdx:jdx + 1],
                                                     scale=ISCALE)
                                if mul_mask and "nomask" not in ablate:
                                    eng = gp_mul
                                    for hp in (0, 1):
                                        eng.tensor_mul(out=et[:, hp, :wj],
                                                       in0=et[:, hp, :wj],
                                                       in1=mask_sb[:, jdx, :wj])
                                e_tiles[j] = et

                        pg_tiles = {}

                        def emit_av(hp, h, g):
                            g0 = HALF * g
                            if g == 0:
                                pg_tiles[hp] = pg_ps.tile([65, NG], F32,
                                                          tag="pg", name="pg")
                            pgp = pg_tiles[hp]
                            pav = av_ps.tile([128, HALF], F32, tag="av",
                                             name="av")
                            nc.tensor.matmul(
                                pav[:65],
                                vg2[hp * 32:hp * 32 + 32, h * 65:h * 65 + 65],
                                eg[hp * 32:hp * 32 + 32, g0:g0 + HALF],
                                start=True, stop=False)
                            for j in range(4 * g - 1, 4 * g + 5):
                                jdx = j + 1
                                q_lo, wj = WIN[j]
                                lo = max(q_lo, g0)
                                hi = min(q_lo + wj, g0 + HALF)
                                et = e_tiles[j]
                                nc.tensor.matmul(
                                    pav[:65, lo - g0:hi - g0],
                                    v_sb[jdx][:, h * 65:h * 65 + 65],
                                    et[:, hp, lo - q_lo:hi - q_lo],
                                    start=False, stop=(j == 4 * g + 4))
                                if 4 * g <= j <= 4 * g + 3 and \
                                        "nopg" not in ablate:
                                    nc.tensor.matmul(
                                        pgp[:],
                                        v_sb[jdx][:, h * 65:h * 65 + 65],
                                        et[:, hp, wj:wj + NG],
                                        start=(j == 0), stop=(j == NCH - 1))
                            # normalize: 1/den, DRAM round-trip broadcast
                            # (keeps the per-column broadcast off the
                            # in-order compute engines), psum*sbuf mul to ctx
                            rcp = rcsb.tile([1, HALF], F32, tag="rcp",
                                            name="rcp")
                            with nc.allow_low_precision(reason="recip"):
                                nc.vector.reciprocal(out=rcp[:],
                                                     in_=pav[64:65, :])
                            if "nonorm" not in ablate:
                                ridx = (p * 2 + hp) * 2 + g
                                nc.sync.dma_start(
                                    out=ap["scr"][ridx:ridx + 1, :], in_=rcp[:])
                                bc = bcsb.tile([64, HALF], F32, tag="bc",
                                               name="bc")
                                gp_dma.dma_start(
                                    out=bc,
                                    in_=ap["scr"][ridx].unsqueeze(0)
                                    .partition_broadcast(64))
                            csl = ctx_raw[p][(h % 2) * DH:(h % 2) * DH + DH,
                                             g0:g0 + HALF]
                            ceng = nc.gpsimd if os.environ.get(
                                "K2_CTXPOOL") else nc.vector
                            if "nonorm" in ablate:
                                ceng.tensor_copy(out=csl, in_=pav[0:64, :])
                            else:
                                ceng.tensor_mul(out=csl, in0=pav[0:64, :],
                                                in1=bc)
                            if g == 1:
                                ogt = ogsb.tile([65, NG], F32, tag="og",
                                                name="og")
                                nc.vector.tensor_copy(out=ogt, in_=pgp)
                                nc.sync.dma_start(out=ap["og"][h], in_=ogt)

                        # interleave scores and AV so the in-order PE stream
                        # always has matmuls between the ACT/DVE round-trips
                        emit_scores(range(-1, 5))     # jdx 0..5
                        emit_av(0, h0, 0)
                        emit_scores(range(5, 8))      # jdx 6..8
                        emit_av(1, h1, 0)
                        emit_scores(range(8, NCH + 1))  # jdx 9
                        emit_av(0, h0, 1)
                        emit_av(1, h1, 1)

        # ---------------- Wo + residual + LN1 ----------------
        if stop_after == "attn":
            dummy_out(pool_ctx, og_too=False)
            return
        with tc.tile_pool(name="wo_pool", bufs=1) as wo_pool, \
             tc.tile_pool(name="usq_sb", bufs=1) as usq_sb, \
             tc.tile_pool(name="row_sb", bufs=2) as row_sb, \
             tc.tile_pool(name="bcast_sb", bufs=2) as bcast_sb, \
             tc.tile_pool(name="s_ps", bufs=2, space="PSUM") as s_ps, \
             tc.tile_pool(name="bc_ps", bufs=1, space="PSUM") as bc_ps, \
             tc.tile_pool(name="u_sb", bufs=1) as u_sb, \
             tc.tile_pool(name="wo_ps", bufs=2, space="PSUM") as wo_ps:
            ln_pools = (usq_sb, row_sb, bcast_sb, s_ps, bc_ps)
            for k in range(KD):
                wo.append(wo_pool.tile([128, D], BF16, tag=f"wo{k}",
                                       name=f"wo{k}"))
                nc.sync.dma_start(out=wo[k],
                                  in_=ap["Wo"][k * 128:(k + 1) * 128, :])
            for c0 in range(0, T, HALF):
                u_tiles = []
                for o in range(KD):
                    ps = wo_ps.tile([128, HALF], F32, tag="wops",
                                    name="wops")
                    for k in range(KD):
                        nc.tensor.matmul(ps[:],
                                         wo[k][:, o * 128:(o + 1) * 128],
                                         ctx_raw[k][:, c0:c0 + HALF],
                                         start=(k == 0), stop=False)
                    nc.tensor.matmul(ps[:], identB,
                                     xT[o][:, W + c0:W + c0 + HALF],
                                     start=False, stop=True)
                    u = u_sb.tile([128, HALF], BF16, tag=f"u{o}",
                                  name=f"u{o}")
                    nc.scalar.activation(out=u, in_=ps, func=AF.Identity,
                                         bias=bo_sb[:, o:o + 1])
                    u_tiles.append(u)
                layernorm(u_tiles, HALF, g1_sb, b1_sb,
                          [attn_outB[(k, c0)][:] for k in range(KD)],
                          ln_pools)

    # ---------------- FFN (two passes: inter tiles staged in SBUF) --------
    if stop_after == "wo":
        dummy_out(pool_ao, og_too=False)
        return
    with tc.tile_pool(name="usq2_sb", bufs=1) as usq_sb, \
         tc.tile_pool(name="row2_sb", bufs=2) as row_sb, \
         tc.tile_pool(name="bcast2_sb", bufs=2) as bcast_sb, \
         tc.tile_pool(name="ffn_ps", bufs=2, space="PSUM") as ffn_ps, \
         tc.tile_pool(name="o2_ps", bufs=2, space="PSUM") as o2_ps, \
         tc.tile_pool(name="s2_ps", bufs=2, space="PSUM") as s_ps, \
         tc.tile_pool(name="bc2_ps", bufs=1, space="PSUM") as bc_ps, \
         tc.tile_pool(name="u2_sb", bufs=1) as u2_sb, \
         tc.tile_pool(name="it_sb", bufs=1) as it_sb, \
         tc.tile_pool(name="out_sb", bufs=2) as out_sb:
        ln_pools = (usq_sb, row_sb, bcast_sb, s_ps, bc_ps)
        if True:
                for c0 in range(0, T, HALF):
                    its = []
                    for f in range(KF):
                        ps = ffn_ps.tile([128, HALF], F32, tag="ffn",
                                         name="ffn")
                        for k in range(KD):
                            nc.tensor.matmul(
                                ps[:], wi[k][:, f * 128:(f + 1) * 128],
                                attn_outB[(k, c0)][:],
                                start=(k == 0), stop=(k == KD - 1))
                        it = it_sb.tile([128, HALF], BF16, tag=f"it{f}",
                                        name=f"it{f}")
                        nc.scalar.activation(out=it, in_=ps, func=gelu_f,
                                             bias=bi_sb[:, f:f + 1])
                        its.append(it)
                    u2_tiles = []
                    for o in range(KD):
                        ps = o2_ps.tile([128, HALF], F32, tag="o2", name="o2")
                        for f in range(KF):
                            nc.tensor.matmul(ps[:],
                                             wo2[f][:, o * 128:(o + 1) * 128],
                                             its[f], start=(f == 0),
                                             stop=False)
                        nc.tensor.matmul(ps[:], identB,
                                         attn_outB[(o, c0)][:],
                                         start=False, stop=True)
                        u2 = u2_sb.tile([128, HALF], BF16, tag=f"u2_{o}",
                                        name=f"u2_{o}")
                        nc.scalar.activation(out=u2, in_=ps, func=AF.Identity,
                                             bias=bo2_sb[:, o:o + 1])
                        u2_tiles.append(u2)
                    dest = [out_sb.tile([128, HALF], F32, tag=f"ot{k}",
                                        name=f"ot{k}") for k in range(KD)]
                    qw = 256 if c0 == HALF else HALF
                    for q0 in range(0, HALF, qw):
                        layernorm([u2_tiles[k][:, q0:q0 + qw]
                                   for k in range(KD)],
                                  qw, g2_sb, b2_sb,
                                  [dest[k][:, q0:q0 + qw] for k in range(KD)],
                                  ln_pools)
                        for k in range(KD):
                            nc.sync.dma_start(
                                out=ap["outT"][k * 128:(k + 1) * 128,
                                               c0 + q0:c0 + q0 + qw],
                                in_=dest[k][:, q0:q0 + qw])


def build_nc(sim_mode=False, repeat=1):
    from contextlib import ExitStack
    nc = bacc.Bacc("TRN2", target_bir_lowering=False, debug=False)
    ap = {}
    ap["xT"] = nc.dram_tensor("xT", [D, TH], BF16, kind="ExternalInput").ap()
    ap["xgT"] = nc.dram_tensor("xgT", [D, NG], BF16, kind="ExternalInput").ap()
    ap["am_halo"] = nc.dram_tensor("am_halo", [TH], F32, kind="ExternalInput").ap()
    ap["am_glob2"] = nc.dram_tensor("am_glob2", [64], F32, kind="ExternalInput").ap()
    ap["mask_all"] = nc.dram_tensor("mask_all", [128, NJ, 3 * W], BF16,
                                    kind="ExternalInput").ap()
    for n, sh in (("Wq", [D, D]), ("Wk", [D, D]), ("Wv", [D, D]), ("Wo", [D, D]),
                  ("Wi", [D, FF]), ("Wo2", [FF, D])):
        ap[n] = nc.dram_tensor(n, sh, BF16, kind="ExternalInput").ap()
    for n, sh in (("bq", [D]), ("bk", [D]), ("bv", [D]), ("bo", [D]),
                  ("bi", [FF]), ("bo2", [D]), ("ln1_g", [D]), ("ln1_b", [D]),
                  ("ln2_g", [D]), ("ln2_b", [D])):
        ap[n] = nc.dram_tensor(n, sh, F32, kind="ExternalInput").ap()
    ap["outT"] = nc.dram_tensor("outT", [D, T], F32, kind="ExternalOutput").ap()
    ap["og"] = nc.dram_tensor("og", [H, 65, NG], F32, kind="ExternalOutput").ap()
    # DRAM scratch for softmax-denominator broadcast round-trips
    ap["scr"] = nc.dram_tensor("scr", [24, HALF], F32).ap()

    with tile.TileContext(nc) as tc:
        if repeat > 1:
            import os
            mu = int(os.environ.get("K2_UNROLL", "1"))
            def body(i):
                with ExitStack() as c2:
                    build_body(nc, tc, ap, c2, sim_mode)
            tc.For_i_unrolled(0, repeat, 1, body, max_unroll=mu)
        else:
            with ExitStack() as c2:
                build_body(nc, tc, ap, c2, sim_mode)
    nc.compile()
    return nc


# ---------------- host side ----------------

def shard_inputs(inputs):
    hs = np.asarray(inputs["hidden_states"], np.float32)
    am = np.asarray(inputs["attention_mask"], np.float32)
    shared = {}
    for n in ("Wq", "bq", "Wk", "bk", "Wv", "bv", "Wo", "bo", "ln1_g", "ln1_b",
              "Wi", "bi", "Wo2", "bo2", "ln2_g", "ln2_b"):
        shared[n] = np.ascontiguousarray(np.asarray(inputs[n], np.float32))
    for n in ("Wq", "Wk", "Wv", "Wo", "Wi", "Wo2"):
        shared[n] = shared[n].astype(ml_dtypes.bfloat16)
    in_maps = []
    for core in range(N_CORES):
        b, q0 = core // 4, (core % 4) * T
        xh = np.zeros((TH, D), np.float32)
        amh = np.zeros((TH,), np.float32)
        lo, hi = q0 - W, q0 + T + W
        slo, shi = max(lo, 0), min(hi, S)
        xh[slo - lo:shi - lo] = hs[b, slo:shi]
        amh[slo - lo:shi - lo] = am[b, slo:shi]
        # additive pre-exp mask: 0 where valid, -1000 where invalid (folded
        # into the score psum via an identity matmul; exp(-125) -> 0).
        # K2_MULMASK=1 switches back to the post-exp 0/1 multiply.
        import os
        mm = os.environ.get("K2_MULMASK", "1") == "1"
        mask = np.full((NJ, 128, 3 * W), 0.0 if mm else -1000.0, np.float32)
        for j in range(-1, NCH + 1):
            cs = [c for c in (j - 1, j, j + 1) if 0 <= c < NCH]
            kpos = q0 + j * 128 + np.arange(128)[:, None]
            for i, c in enumerate(cs):
                qpos = q0 + c * 128 + np.arange(128)[None, :]
                valid = (np.abs(kpos - qpos) <= W) & (kpos >= NG) & (kpos >= 0) \
                    & (kpos < S)
                mask[j + 1, :, i * 128:(i + 1) * 128] = \
                    valid if mm else np.where(valid, 0.0, -1000.0)
        amg = np.ascontiguousarray(am[b, :NG])
        m = {"xT": np.ascontiguousarray(xh.T).astype(ml_dtypes.bfloat16),
             "xgT": np.ascontiguousarray(hs[b, :NG].T).astype(ml_dtypes.bfloat16),
             "am_halo": amh,
             "am_glob2": np.concatenate([amg, amg]),
             "mask_all": np.ascontiguousarray(
                 mask.transpose(1, 0, 2)).astype(ml_dtypes.bfloat16)}
        m.update(shared)
        in_maps.append(m)
    return in_maps


def _np_layernorm(x, g, b):
    mu = x.mean(-1, keepdims=True)
    var = ((x - mu) ** 2).mean(-1, keepdims=True)
    return (x - mu) / np.sqrt(var + EPS) * g + b


def _np_gelu(x):
    from scipy.special import erf
    return x * 0.5 * (1.0 + erf(x / np.sqrt(2.0)))


def host_tail(inputs, og_by_core, sim_mode=False):
    """Combine global-query flash partials; dense tail for the global rows."""
    hs = np.asarray(inputs["hidden_states"], np.float64)
    rows = np.zeros((B, NG, D))
    for b in range(B):
        o = sum(np.asarray(og_by_core[4 * b + c], np.float64) for c in range(4))
        gctx = o[:, :DH, :] / o[:, 64:65, :]          # [H, DH, NG]
        gctx = gctx.transpose(2, 0, 1).reshape(NG, D)  # feature index = h*64+d
        u = gctx @ np.asarray(inputs["Wo"], np.float64) \
            + np.asarray(inputs["bo"], np.float64) + hs[b, :NG]
        a = _np_layernorm(u, np.asarray(inputs["ln1_g"], np.float64),
                          np.asarray(inputs["ln1_b"], np.float64))
        inter = a @ np.asarray(inputs["Wi"], np.float64) \
            + np.asarray(inputs["bi"], np.float64)
        if not sim_mode:
            inter = _np_gelu(inter)
        u2 = inter @ np.asarray(inputs["Wo2"], np.float64) \
            + np.asarray(inputs["bo2"], np.float64) + a
        rows[b] = _np_layernorm(u2, np.asarray(inputs["ln2_g"], np.float64),
                                np.asarray(inputs["ln2_b"], np.float64))
    return rows.astype(np.float32)


def assemble(inputs, results, sim_mode=False):
    out = np.zeros((B, S, D), np.float32)
    for core in range(N_CORES):
        b, q0 = core // 4, (core % 4) * T
        out[b, q0:q0 + T] = np.asarray(results[core]["outT"]).T
    out[:, :NG] = host_tail(inputs, [results[c]["og"] for c in range(N_CORES)],
                            sim_mode)
    return out


def kernel(**inputs):
    from concourse import bass_utils
    if "nc" not in _nc_cache:
        _nc_cache["nc"] = build_nc()
    nc = _nc_cache["nc"]
    in_maps = shard_inputs(inputs)
    res = bass_utils.run_bass_kernel_spmd(nc, in_maps, core_ids=list(range(N_CORES)))
    return assemble(inputs, res.results)

